# revision 5
# baseline (speedup 1.0000x reference)
"""CARE-GNN forward on 8 Trainium2 NeuronCores (Bass/Tile), v4.

Strategy (dst-sharded, compact edge chunks, 256B table rows):
  - Nodes are sharded across 8 cores by dst range; each core owns all edges
    into its nodes, so segment sums/counts complete locally.
  - Per layer every core reads a full node table from HBM.  Table rows are
    256B: r = h @ M where M is identity with column p* (p* = argmax|wj|)
    replaced by wj — so r[p*] = h.wj = pj rides inside the 128-wide row and
    the true h is recovered after aggregation by one matmul with M^-1
    (aggregation is linear in the row).  256B rows halve gather SBUF-write
    traffic, the AllGathers, and the table footprint vs [h|pj|pad] 512B.
  - The table is split in two halves by slab row (tiles 0..24 -> region A,
    25..48 -> region B), each rebuilt by its own AllGather; region-A gathers
    start while the region-B collective is in flight, and both index spaces
    stay < 2^15 for signed-int16 dma_gather addressing.
  - Edges are packed DENSELY per (dst tile, src region): 128 per chunk; pad
    slots carry idx -1 (trailing negatives are trimmed by the gather ucode)
    and dstpos 255 so their scatter-matrix row is zero.
  - Aggregation per chunk k: acc[pos,:] += S_k^T @ g_k in PSUM, with
    S_k[e,pos] = alpha_e at pos==dstpos_e else 0, built on device:
    S0 = is_equal(dstpos, IOTA) (one DVE op per tile), SIG via one Act op
    per chunk (in_ = per-tile broadcast matrix PIB bf16, bias = gathered pj
    column), S = S0 * SIG (one DVE op per tile).
  - dma_gather calls (~2k rows each) rotate over 4 SWDGE queues.
"""

import os
import sys
import types

import numpy as np
import ml_dtypes

N_CORES = 8
HID = 128
ROW_W = HID  # bf16 elements per table row (256B): r = h @ M
ATILES = 25  # slab tiles in region A (B gets tpc - ATILES)


def _install_axon_ntff_hook():
    """Best-effort shim so trace=True (BASS_TRACE=1) works under axon."""
    try:
        if "antenv.axon_hooks" in sys.modules:
            return
        mod = types.ModuleType("antenv.axon_hooks")
        mod._hook = None
        mod.set_axon_ntff_profile_hook = lambda h: setattr(mod, "_hook", h)
        mod.get_axon_ntff_profile_hook = lambda: mod._hook
        sys.modules["antenv.axon_hooks"] = mod
        import antenv

        antenv.axon_hooks = mod
        from trn_agent_boot.trn_boot import _ntff_profile_via_ctypes

        so = "/opt/axon/libaxon_pjrt.so"
        if os.path.exists(so):
            mod.set_axon_ntff_profile_hook(_ntff_profile_via_ctypes(so))
    except Exception:
        pass


def _host_prep(x, edge_index):
    """Shard nodes/edges, build compact-chunk gather indices. Pure index work."""
    N = x.shape[0]
    src = np.asarray(edge_index[0], dtype=np.int64)
    dst = np.asarray(edge_index[1], dtype=np.int64)
    npc = (N + N_CORES - 1) // N_CORES
    tpc = (npc + 127) // 128
    slab = tpc * 128
    arows = ATILES * 128
    brows = slab - arows

    deg = np.bincount(dst, minlength=N)
    owner = np.minimum(np.arange(N) // npc, N_CORES - 1)
    slabrow = np.arange(N) - owner * npc

    e_core = owner[dst]
    e_slabrow = slabrow[dst]
    e_tile = e_slabrow // 128
    e_pos = e_slabrow % 128
    s_slabrow = slabrow[src]
    e_reg = (s_slabrow >= arows).astype(np.int64)
    e_srcrow = np.where(
        e_reg == 0,
        owner[src] * arows + s_slabrow,
        owner[src] * brows + (s_slabrow - arows),
    )

    gid = (e_core * tpc + e_tile) * 2 + e_reg
    order = np.argsort(gid * 128 + e_pos, kind="stable")
    gs = gid[order]
    grp_start = np.r_[0, np.flatnonzero(np.diff(gs)) + 1]
    grp_len = np.diff(np.r_[grp_start, len(gs)])
    seq = np.arange(len(gs)) - np.repeat(grp_start, grp_len)
    e_seq = np.empty(len(gs), dtype=np.int64)
    e_seq[order] = seq

    cnt = np.zeros((N_CORES, tpc, 2), dtype=np.int64)
    np.add.at(cnt, (e_core, e_tile, e_reg), 1)
    nch = ((cnt + 127) // 128).max(axis=0)  # [tpc, 2] shared across cores
    CA = nch[:, 0].astype(int)
    CB = nch[:, 1].astype(int)
    CT = CA + CB
    assert CA.min() >= 1 and CB.min() >= 1

    offs_a = np.r_[0, np.cumsum(CA)].astype(int)
    offs_b = np.r_[0, np.cumsum(CB)].astype(int)
    offs_t = np.r_[0, np.cumsum(CT)].astype(int)
    tot_a, tot_b, tot_t = int(offs_a[-1]), int(offs_b[-1]), int(offs_t[-1])

    # pad slots: idx -1 (ucode trims trailing negatives), dstpos 255 (S0 row 0)
    idx_a = np.zeros((N_CORES, tot_a * 128), dtype=np.int64)
    idx_b = np.zeros((N_CORES, tot_b * 128), dtype=np.int64)
    dstp = np.full((N_CORES, 128, tot_t), 255.0, dtype=np.float32)

    e_ch = e_seq // 128
    e_lane = e_seq % 128
    mA = e_reg == 0
    mB = ~mA
    linA = (offs_a[e_tile[mA]] + e_ch[mA]) * 128 + e_lane[mA]
    linB = (offs_b[e_tile[mB]] + e_ch[mB]) * 128 + e_lane[mB]
    idx_a[e_core[mA], linA] = e_srcrow[mA]
    idx_b[e_core[mB], linB] = e_srcrow[mB]
    dstp[e_core[mA], e_lane[mA], offs_t[e_tile[mA]] + e_ch[mA]] = e_pos[mA]
    dstp[e_core[mB], e_lane[mB], offs_t[e_tile[mB]] + CA[e_tile[mB]] + e_ch[mB]] = (
        e_pos[mB]
    )

    def wrap16(lin):  # [n] -> [128, n//16] int16 (16-part wrap, replicated x8)
        w = lin.reshape(-1, 16).T.astype(np.uint16).view(np.int16)
        return np.tile(w, (8, 1))

    idx_a16 = np.stack([wrap16(idx_a[c]) for c in range(N_CORES)])
    idx_b16 = np.stack([wrap16(idx_b[c]) for c in range(N_CORES)])

    invdeg = np.ones((N_CORES, slab), dtype=np.float32)
    invdeg[owner, slabrow] = 1.0 / np.maximum(deg, 1).astype(np.float32)
    invdeg = invdeg.reshape(N_CORES, tpc, 128).transpose(0, 2, 1).copy()

    in_dim = x.shape[1]
    xT = np.zeros((N_CORES, in_dim, slab), dtype=ml_dtypes.bfloat16)
    for c in range(N_CORES):
        lo, hi = c * npc, min((c + 1) * npc, N)
        xT[c][:, slabrow[lo:hi]] = (
            np.asarray(x[lo:hi], dtype=np.float32).astype(ml_dtypes.bfloat16).T
        )

    return dict(
        N=N, npc=npc, tpc=tpc, slab=slab, arows=arows, brows=brows,
        in_dim=in_dim,
        CA=CA, CB=CB, offs_a=offs_a, offs_b=offs_b, offs_t=offs_t,
        tot_a=tot_a, tot_b=tot_b, tot_t=tot_t,
        idx_a16=idx_a16, idx_b16=idx_b16, dstp=dstp, invdeg=invdeg, xT=xT,
        owner=owner, slabrow=slabrow,
    )


def _build_program(p, consts):
    import concourse.bacc as bacc
    import concourse.mybir as mybir
    import concourse.tile as tile

    f32 = mybir.dt.float32
    bf16 = mybir.dt.bfloat16
    i16 = mybir.dt.int16
    AF = mybir.ActivationFunctionType

    tpc, slab, in_dim = p["tpc"], p["slab"], p["in_dim"]
    arows, brows = p["arows"], p["brows"]
    CA, CB = p["CA"], p["CB"]
    offs_a, offs_b, offs_t = p["offs_a"], p["offs_b"], p["offs_t"]
    tot_a, tot_b, tot_t = p["tot_a"], p["tot_b"], p["tot_t"]
    att_bs = (consts["att1_b"], consts["att2_b"])
    pstar = consts["pstar"]  # pivot column per layer
    nk = in_dim // 128
    STRIP = 8

    nc = bacc.Bacc("TRN2", num_devices=N_CORES, num_swdge_queues=4,
                   dynamic_dma_scratch_size=65536)

    # ---- I/O ----
    xT = nc.dram_tensor("xT", [in_dim, slab], bf16, kind="ExternalInput")
    idxA = nc.dram_tensor("idxA", [128, tot_a * 8], i16, kind="ExternalInput")
    idxB = nc.dram_tensor("idxB", [128, tot_b * 8], i16, kind="ExternalInput")
    dstpT = nc.dram_tensor("dstpT", [128, tot_t], bf16, kind="ExternalInput")
    invdeg = nc.dram_tensor("invdeg", [128, tpc], f32, kind="ExternalInput")
    encw = nc.dram_tensor("encw", [in_dim, HID], bf16, kind="ExternalInput")
    encb = nc.dram_tensor("encb", [HID, 1], f32, kind="ExternalInput")
    m1 = nc.dram_tensor("m1", [HID, HID], bf16, kind="ExternalInput")
    m2 = nc.dram_tensor("m2", [HID, HID], bf16, kind="ExternalInput")
    m1inv = nc.dram_tensor("m1inv", [HID, HID], bf16, kind="ExternalInput")
    wi2c = nc.dram_tensor("wi2c", [HID, 2], bf16, kind="ExternalInput")
    wout = nc.dram_tensor("wout", [HID, 2], f32, kind="ExternalInput")
    clsb = nc.dram_tensor("clsb", [1, 2], f32, kind="ExternalInput")
    ident_in = nc.dram_tensor("ident", [128, 128], f32, kind="ExternalInput")
    iota_in = nc.dram_tensor("iota", [128, 128], bf16, kind="ExternalInput")
    logits = nc.dram_tensor("logits", [slab, 2], f32, kind="ExternalOutput")

    # ---- internal DRAM ----
    slabs = [nc.dram_tensor(f"slab{l}", [slab, ROW_W], bf16) for l in (1, 2)]
    tabsA = [
        nc.dram_tensor(f"tabA{l}", [N_CORES * arows, ROW_W], bf16,
                       addr_space="Shared")
        for l in (1, 2)
    ]
    tabsB = [
        nc.dram_tensor(f"tabB{l}", [N_CORES * brows, ROW_W], bf16,
                       addr_space="Shared")
        for l in (1, 2)
    ]

    with tile.TileContext(nc) as tc:
        with (
            tc.tile_pool(name="const", bufs=1) as cpool,
            tc.tile_pool(name="work", bufs=3) as pool,
            tc.tile_pool(name="smat", bufs=2) as spool,
            tc.tile_pool(name="gath", bufs=6) as gpool,
            tc.tile_pool(name="psacc", bufs=2, space="PSUM") as ps_acc,
            tc.tile_pool(name="pstr", bufs=2, space="PSUM") as ps_tr,
            tc.tile_pool(name="pssm", bufs=2, space="PSUM") as ps_sm,
        ):
            # ---- constants ----
            encw_t = [cpool.tile([128, HID], bf16, tag=f"encw{i}", name=f"encw{i}") for i in range(nk)]
            for i, t in enumerate(encw_t):
                nc.sync.dma_start(out=t[:], in_=encw[i * 128 : (i + 1) * 128, :])
            encb_t = cpool.tile([HID, 1], f32, tag="encb")
            nc.sync.dma_start(out=encb_t[:], in_=encb[:])
            m_t = [cpool.tile([HID, HID], bf16, tag=f"m{l}", name=f"m{l}") for l in (1, 2)]
            nc.sync.dma_start(out=m_t[0][:], in_=m1[:])
            nc.sync.dma_start(out=m_t[1][:], in_=m2[:])
            m1inv_t = cpool.tile([HID, HID], bf16, tag="m1inv")
            nc.sync.dma_start(out=m1inv_t[:], in_=m1inv[:])
            wi2c_t = cpool.tile([HID, 2], bf16, tag="wi2c")
            nc.sync.dma_start(out=wi2c_t[:], in_=wi2c[:])
            wout_t = cpool.tile([HID, 2], f32, tag="wout")
            nc.sync.dma_start(out=wout_t[:], in_=wout[:])
            clsb_t = cpool.tile([1, 2], f32, tag="clsb")
            nc.sync.dma_start(out=clsb_t[:], in_=clsb[:])
            ident_f = cpool.tile([128, 128], f32, tag="identf")
            nc.sync.dma_start(out=ident_f[:], in_=ident_in[:])
            iota_b = cpool.tile([128, 128], bf16, tag="iotab")
            nc.sync.dma_start(out=iota_b[:], in_=iota_in[:])
            ones_f = cpool.tile([1, 128], f32, tag="onesf")
            nc.vector.memset(ones_f[:], 1.0)
            inv_all = cpool.tile([128, tpc], f32, tag="invall")
            nc.sync.dma_start(out=inv_all[:], in_=invdeg[:])
            idxA_t = cpool.tile([128, tot_a * 8], i16, tag="idxAt")
            nc.sync.dma_start(out=idxA_t[:], in_=idxA[:])
            idxB_t = cpool.tile([128, tot_b * 8], i16, tag="idxBt")
            nc.sync.dma_start(out=idxB_t[:], in_=idxB[:])
            dstp_all = cpool.tile([128, tot_t], bf16, tag="dstpall")
            nc.sync.dma_start(out=dstp_all[:], in_=dstpT[:])
            pi_all = [
                cpool.tile([128, tpc], f32, tag=f"piall{l}", name=f"piall{l}")
                for l in (1, 2)
            ]

            def p_phase_and_store(hT_sb, t, layer):
                """hT (bf16 [hid, nodes]) -> slab rows r = h@M + pi' column."""
                co = t * 128
                r_ps = ps_tr.tile([128, 128], f32, tag="tr")
                nc.tensor.matmul(
                    out=r_ps[:], lhsT=hT_sb[:], rhs=m_t[layer - 1][:],
                    start=True, stop=True,
                )
                r_sb = pool.tile([128, 128], bf16, tag="rsb")
                nc.vector.tensor_copy(out=r_sb[:], in_=r_ps[:])
                nc.sync.dma_start(out=slabs[layer - 1][co : co + 128, :], in_=r_sb[:])
                p_ps = ps_sm.tile([128, 1], f32, tag="sm")
                nc.tensor.matmul(
                    out=p_ps[:], lhsT=hT_sb[:],
                    rhs=wi2c_t[:, layer - 1 : layer],
                    start=True, stop=True,
                )
                nc.scalar.add(
                    out=pi_all[layer - 1][:, t : t + 1],
                    in_=p_ps[:, 0:1],
                    add=float(att_bs[layer - 1]),
                )

            def all_gather(reg, layer):
                tab = (tabsA if reg == 0 else tabsB)[layer - 1]
                lo = 0 if reg == 0 else arows
                hi = arows if reg == 0 else slab
                nc.gpsimd.collective_compute(
                    "AllGather",
                    mybir.AluOpType.bypass,
                    replica_groups=[list(range(N_CORES))],
                    ins=[slabs[layer - 1][lo:hi, :]],
                    outs=[tab[:]],
                )

            # ---- encoder ----
            for t in range(tpc):
                if t % STRIP == 0:
                    ns = min(STRIP, tpc - t) * 128
                    xs = [
                        pool.tile([128, STRIP * 128], bf16, tag=f"xs{i}", name=f"xs{i}")
                        for i in range(nk)
                    ]
                    for i, xx in enumerate(xs):
                        nc.sync.dma_start(
                            out=xx[:, 0:ns],
                            in_=xT[i * 128 : (i + 1) * 128, t * 128 : t * 128 + ns],
                        )
                so = (t % STRIP) * 128
                hT_ps = ps_tr.tile([128, 128], f32, tag="tr")
                for i in range(nk):
                    nc.tensor.matmul(
                        out=hT_ps[:], lhsT=encw_t[i][:], rhs=xs[i][:, so : so + 128],
                        start=(i == 0), stop=(i == nk - 1),
                    )
                hT_sb = pool.tile([128, 128], bf16, tag="hTsb")
                nc.scalar.activation(
                    out=hT_sb[:], in_=hT_ps[:], func=AF.Relu, bias=encb_t[:]
                )
                p_phase_and_store(hT_sb, t, layer=1)
                if t == ATILES - 1:
                    all_gather(0, 1)  # region-A table; overlaps encoder tail

            # ---- two message-passing layers ----
            gather_ctr = [0]
            fresh = {"g0": 0, "g1": 0}
            GBUFS = 6

            def gather(reg, t, layer):
                cr = int((CA if reg == 0 else CB)[t])
                offs = offs_a if reg == 0 else offs_b
                idx_t = idxA_t if reg == 0 else idxB_t
                tab = (tabsA if reg == 0 else tabsB)[layer - 1]
                gt = gpool.tile([128, cr, ROW_W], bf16, tag=f"g{reg}", name=f"g{reg}")
                if fresh[f"g{reg}"] < GBUFS:
                    # first use of this buffer: clear so trimmed pad lanes
                    # hold finite values (S0 zero-rows nuke them later)
                    nc.vector.memset(gt[:], 0.0)
                    fresh[f"g{reg}"] += 1
                nc.gpsimd.dma_gather(
                    out_ap=gt[:],
                    in_ap=tab[:, :],
                    idxs_ap=idx_t[:, offs[t] * 8 : (offs[t] + cr) * 8],
                    num_idxs=cr * 128,
                    num_idxs_reg=cr * 128,
                    elem_size=ROW_W,
                    single_packet=False,
                    queue_num=gather_ctr[0] % 4,
                )
                gather_ctr[0] += 1
                return gt

            WARM = 5

            for layer in (1, 2):
                ps = pstar[layer - 1]
                pend = []
                for t in range(WARM):
                    pend.append([t, gather(0, t, layer)])
                # region-B collective AFTER the warm region-A gathers so
                # their descriptor generation overlaps the previous phase
                all_gather(1, layer)

                for t in range(tpc):
                    if t + WARM < tpc:
                        pend.append([t + WARM, gather(0, t + WARM, layer)])
                    ent = pend.pop(0)
                    assert ent[0] == t
                    g = [ent[1], gather(1, t, layer)]
                    ca, cb = int(CA[t]), int(CB[t])
                    ct = ca + cb
                    co = t * 128
                    # S0[e, k, pos] = (dstpos[e, k] == pos)
                    s0 = spool.tile([128, ct, 128], bf16, tag="s0")
                    nc.vector.tensor_tensor(
                        out=s0[:],
                        in0=dstp_all[:, offs_t[t] : offs_t[t] + ct, None].to_broadcast(
                            [128, ct, 128]
                        ),
                        in1=iota_b[:, None, :].to_broadcast([128, ct, 128]),
                        op=mybir.AluOpType.is_equal,
                    )
                    # PIB[e, pos] = pi'[pos] (bf16) via two tiny PE matmuls
                    pirow_ps = ps_sm.tile([1, 128], f32, tag="pirow", bufs=1)
                    nc.tensor.matmul(
                        out=pirow_ps[:], lhsT=pi_all[layer - 1][:, t : t + 1],
                        rhs=ident_f[:], start=True, stop=True,
                    )
                    pirow_sb = pool.tile([1, 128], f32, tag="pirowsb")
                    nc.vector.tensor_copy(out=pirow_sb[:], in_=pirow_ps[:])
                    pib_ps = ps_tr.tile([128, 128], f32, tag="pib", bufs=1)
                    nc.tensor.matmul(
                        out=pib_ps[:], lhsT=ones_f[:], rhs=pirow_sb[:],
                        start=True, stop=True,
                    )
                    pib_sb = pool.tile([128, 128], bf16, tag="pibsb")
                    nc.vector.tensor_copy(out=pib_sb[:], in_=pib_ps[:])
                    # SIG per chunk (Act), then one big mask multiply (DVE)
                    smat = spool.tile([128, ct, 128], bf16, tag="smat")
                    for k in range(ct):
                        reg, c = (0, k) if k < ca else (1, k - ca)
                        nc.scalar.activation(
                            out=smat[:, k, :],
                            in_=pib_sb[:],
                            func=AF.Sigmoid,
                            bias=g[reg][:, c, ps : ps + 1],
                        )
                    nc.vector.tensor_tensor(
                        out=smat[:], in0=smat[:], in1=s0[:],
                        op=mybir.AluOpType.mult,
                    )
                    acc = ps_acc.tile([128, HID], f32, tag="acc")
                    for k in range(ct):
                        reg, c = (0, k) if k < ca else (1, k - ca)
                        nc.tensor.matmul(
                            out=acc[:], lhsT=smat[:, k, :], rhs=g[reg][:, c, :],
                            start=(k == 0), stop=(k == ct - 1),
                        )
                    inv_col = inv_all[:, t : t + 1]
                    if layer == 1:
                        # h2T = relu(M1inv^T @ (inv (.) acc_r)^T)
                        m_sb = pool.tile([128, 128], f32, tag="msb")
                        nc.scalar.mul(out=m_sb[:], in_=acc[:], mul=inv_col)
                        mT_ps = ps_tr.tile([128, 128], f32, tag="tr")
                        nc.tensor.transpose(
                            out=mT_ps[:], in_=m_sb[:], identity=ident_f[:]
                        )
                        mT_sb = pool.tile([128, 128], bf16, tag="mTsb")
                        nc.vector.tensor_copy(out=mT_sb[:], in_=mT_ps[:])
                        h2T_ps = ps_tr.tile([128, 128], f32, tag="tr")
                        nc.tensor.matmul(
                            out=h2T_ps[:], lhsT=m1inv_t[:], rhs=mT_sb[:],
                            start=True, stop=True,
                        )
                        h2T_sb = pool.tile([128, 128], bf16, tag="h2Tsb")
                        nc.scalar.activation(
                            out=h2T_sb[:], in_=h2T_ps[:], func=AF.Relu
                        )
                        p_phase_and_store(h2T_sb, t, layer=2)
                        if t == ATILES - 1:
                            all_gather(0, 2)  # overlaps layer-1 tail
                    else:
                        # logits = (inv (.) acc_r2) @ (M2inv clsw) + clsb
                        m_sb = pool.tile([128, 128], f32, tag="msb")
                        nc.scalar.mul(out=m_sb[:], in_=acc[:], mul=inv_col)
                        mT_ps = ps_tr.tile([128, 128], f32, tag="tr")
                        nc.tensor.transpose(
                            out=mT_ps[:], in_=m_sb[:], identity=ident_f[:]
                        )
                        mT_sb = pool.tile([128, 128], f32, tag="mTsb2")
                        nc.vector.tensor_copy(out=mT_sb[:], in_=mT_ps[:])
                        lg_ps = ps_sm.tile([128, 2], f32, tag="sm")
                        nc.tensor.matmul(
                            out=lg_ps[:], lhsT=mT_sb[:], rhs=wout_t[:],
                            start=True, stop=False,
                        )
                        nc.tensor.matmul(
                            out=lg_ps[:], lhsT=ones_f[:], rhs=clsb_t[:],
                            start=False, stop=True,
                        )
                        lg_sb = pool.tile([128, 2], f32, tag="lgsb")
                        nc.vector.tensor_copy(out=lg_sb[:], in_=lg_ps[:])
                        nc.sync.dma_start(out=logits[co : co + 128, :], in_=lg_sb[:])

    nc.compile()
    return nc


_CACHE = {}


def kernel(**inputs):
    _install_axon_ntff_hook()
    from concourse import bass_utils

    bass_utils.upload_artifacts = lambda tmpdir: tmpdir

    x = np.asarray(inputs["x"], dtype=np.float32)
    edge_index = np.asarray(inputs["edge_index"])
    p = _host_prep(x, edge_index)

    # attention weights: att([x_i, x_j]) = x_i@wi + x_j@wj + b
    wi1 = np.asarray(inputs["att1_w"], dtype=np.float32).reshape(2, HID)[0]
    wj1 = np.asarray(inputs["att1_w"], dtype=np.float32).reshape(2, HID)[1]
    wi2 = np.asarray(inputs["att2_w"], dtype=np.float32).reshape(2, HID)[0]
    wj2 = np.asarray(inputs["att2_w"], dtype=np.float32).reshape(2, HID)[1]
    p1 = int(np.abs(wj1).argmax())
    p2 = int(np.abs(wj2).argmax())
    M1 = np.eye(HID, dtype=np.float64); M1[:, p1] = wj1
    M2 = np.eye(HID, dtype=np.float64); M2[:, p2] = wj2
    M1i = np.linalg.inv(M1)
    M2i = np.linalg.inv(M2)
    wout = (M2i @ np.asarray(inputs["cls_w"], dtype=np.float64)).astype(np.float32)

    consts = dict(
        att1_b=float(np.asarray(inputs["att1_b"]).reshape(-1)[0]),
        att2_b=float(np.asarray(inputs["att2_b"]).reshape(-1)[0]),
        pstar=(p1, p2),
    )
    key = (tuple(p["CA"]), tuple(p["CB"]), consts["att1_b"], consts["att2_b"], p1, p2)
    if key not in _CACHE:
        _CACHE[key] = _build_program(p, consts)
    nc = _CACHE[key]

    bf = ml_dtypes.bfloat16
    iota = np.tile(np.arange(128, dtype=np.float32)[None, :], (128, 1))
    common = dict(
        encw=np.ascontiguousarray(
            np.asarray(inputs["enc_w"], dtype=np.float32).astype(bf)
        ),
        encb=np.asarray(inputs["enc_b"], dtype=np.float32).reshape(HID, 1),
        m1=np.ascontiguousarray(M1.astype(np.float32).astype(bf)),
        m2=np.ascontiguousarray(M2.astype(np.float32).astype(bf)),
        m1inv=np.ascontiguousarray(M1i.astype(np.float32).astype(bf)),
        wi2c=np.ascontiguousarray(
            np.stack([wi1, wi2], axis=1).astype(bf)
        ),
        wout=np.ascontiguousarray(wout),
        clsb=np.asarray(inputs["cls_b"], dtype=np.float32).reshape(1, 2),
        ident=np.eye(128, dtype=np.float32),
        iota=iota.astype(bf),
    )
    in_maps = []
    for c in range(N_CORES):
        in_maps.append(
            dict(
                xT=np.ascontiguousarray(p["xT"][c]),
                idxA=np.ascontiguousarray(p["idx_a16"][c]),
                idxB=np.ascontiguousarray(p["idx_b16"][c]),
                dstpT=np.ascontiguousarray(p["dstp"][c].astype(bf)),
                invdeg=np.ascontiguousarray(p["invdeg"][c]),
                **common,
            )
        )

    res = bass_utils.run_bass_kernel_spmd(nc, in_maps, core_ids=list(range(N_CORES)))
    kernel.last_result = res

    N = p["N"]
    out = np.zeros((N, 2), dtype=np.float32)
    for c in range(N_CORES):
        m = p["owner"] == c
        out[m] = np.asarray(res.results[c]["logits"], dtype=np.float32)[
            p["slabrow"][m]
        ]
    return out
